# revision 5
# baseline (speedup 1.0000x reference)
"""Cross-attention reducer kernel for Trainium2, 8 NeuronCores (SPMD), v3.

Problem (full shapes):
    token_input    [T=8192, L=4096]
    learned_queries[V=4096, I=512]
    w_q [I, I], w_k [L, I], w_v [L, I], w_out [I, L]

    q = learned_queries @ w_q;  k = token_input @ w_k;  v = token_input @ w_v
    per head h (H=8, D=64): attn = softmax(q_h k_h^T / sqrt(D)); out_h = attn @ v_h
    out = concat_h(out_h) @ w_out      -> [V, L]

Sharding: sequence-parallel attention with an all-gather of q and per-head
ReduceScatters of partial softmax sums (flash-style partial-sum reduction).
Each core projects its T/8 token shard with the full w_k/w_v, computes for
ALL 4096 queries x 8 heads the partial numerator sum exp(s) v and partial
denominator sum exp(s) over its local tokens (exp without max subtraction:
logits are O(3)), and per head ReduceScatter-sums the partials so core r
gets its own V/8 query block fully reduced; it normalizes and applies w_out.

v3 changes over the earlier kernel:
  - all inputs host-cast to bf16 (halves HBM traffic; matmul error stays
    ~1e-3, far under the 2e-2 gate); projections run bf16 x bf16.
  - head-pair packing: k^T kept as [128 = two heads' D rows, t]; the odd
    head's score matmul uses base partition 64 (PE tile_position (64,0)),
    removing all k partition-shift DMAs; q^T gathered tiles are loaded once
    per head-pair (persistent, 4 tiles) instead of twice per head.
  - partial u accumulated in SBUF (bf16): th0 drains PSUM->u_loc, th1 adds
    PSUM+u_loc->u_loc; removes the DRAM accumulate-DMA bounce entirely.
    ReduceScatters run on bf16 (half the bytes, earlier completion).
  - per-head finalize as soon as its ReduceScatter lands; the last head's
    reduction is the only exposed one.
  - scores/exp/PV software-pipelined one group deep (PE never sits on an
    exp dependency with an empty queue); exp is the only ACT-engine work.
  - startup: lq/wq land first (q proj), first k-projection runs on t-quarter
    chunks as the token shard streams in.
  - w_out preloaded whole (32KB bf16) mid-kernel; output projection drains
    through 4 PSUM banks back-to-back.
"""

import os

import numpy as np

import concourse.bacc as bacc
import concourse.tile as tile
import concourse.mybir as mybir
from concourse.bass_utils import run_bass_kernel_spmd

F32 = mybir.dt.float32
BF16 = mybir.dt.bfloat16
EXP = mybir.ActivationFunctionType.Exp
MULT = mybir.AluOpType.mult
ADD = mybir.AluOpType.add
EQ = mybir.AluOpType.is_equal

N_CORES = 8
T, L, V, INNER = 8192, 4096, 4096, 512
H, D = 8, 64
M = H // 2             # 4 head-pairs == 128-wide i-blocks
TS = T // N_CORES      # 1024  t-shard per core
QS = V // N_CORES      # 512   query shard per core (= ReduceScatter block)
SCALE = D ** -0.5      # 0.125
VW = D + 1             # 65    v head block width incl. ones column

# diagnostics: BASSK_NO_CC=1 -> replace collectives with local copies
# (wrong data, timing only)
_NO_CC = bool(os.environ.get("BASSK_NO_CC"))


def build_program():
    nc = bacc.Bacc(
        "TRN2", target_bir_lowering=False, debug=False, num_devices=N_CORES
    )

    tok_T = nc.dram_tensor("tok_T", [L, TS], BF16, kind="ExternalInput").ap()
    lq_T = nc.dram_tensor("lq_T", [INNER, QS], BF16, kind="ExternalInput").ap()
    w_q = nc.dram_tensor("w_q", [INNER, INNER], BF16, kind="ExternalInput").ap()
    w_k = nc.dram_tensor("w_k", [L, INNER], BF16, kind="ExternalInput").ap()
    w_v = nc.dram_tensor("w_v", [L, INNER], BF16, kind="ExternalInput").ap()
    w_out = nc.dram_tensor("w_out", [INNER, L], BF16, kind="ExternalInput").ap()
    outT = nc.dram_tensor("outT", [L, QS], F32, kind="ExternalOutput").ap()

    # partition-major DRAM views
    tok_v = tok_T.rearrange("(k p) t -> p k t", p=128)      # [128, 32, 1024]
    lq_v = lq_T.rearrange("(k p) q -> p k q", p=128)        # [128, 4, 512]
    w_q_v = w_q.rearrange("(k p) i -> p k i", p=128)        # [128, 4, 512]
    w_k_v = w_k.rearrange("(k p) i -> p k i", p=128)        # [128, 32, 512]
    w_v_v = w_v.rearrange("(k p) i -> p k i", p=128)        # [128, 32, 512]
    w_out_v = w_out.rearrange("(k p) l -> p k l", p=128)    # [128, 4, 4096]

    with tile.TileContext(nc) as tc:
        with (
            tc.tile_pool(name="persist", bufs=1) as persist,
            tc.tile_pool(name="qa", bufs=2) as qa,
            tc.tile_pool(name="pTp", bufs=3) as pTp,
            tc.tile_pool(name="finp", bufs=2) as finp,
            tc.tile_pool(name="wop", bufs=1) as wop,
            tc.tile_pool(name="dram", bufs=1, space="DRAM") as dram,
        ):
            # ---- persistent SBUF ----
            kTh2 = persist.tile([128, M, TS], BF16, tag="kTh2")      # k^T head-pairs
            v_sb = persist.tile([128, TS // 128, H * VW], BF16, tag="v")
            u_loc = persist.tile([128, 2, 32, VW], BF16, tag="u")    # store staging
            aT_sb = persist.tile([128, 4, QS], BF16, tag="aT")       # attn out^T
            idn = persist.tile([128, 128], BF16, tag="idn")          # PE transpose id
            wo_all = wop.tile([128, 4, L], BF16, tag="wo")           # w_out (bf16)

            # collective buffers
            gq_in = dram.tile([INNER, QS], BF16, tag="gq_in")
            gq_out = dram.tile(
                [N_CORES * INNER, QS], BF16, tag="gq_out", addr_space="Shared"
            )
            u_dram = dram.tile([H, V, VW], BF16, tag="u_dram")
            u_red = dram.tile([H, QS, VW], BF16, tag="u_red")

            # gathered q^T viewed per head-pair: [128, m, c, q]
            gq_view = gq_out.rearrange("(c m p) q -> p m c q", p=128, m=M)

            # identity matrix for PE transposes: idn[p, f] = (f == p)
            with tc.tile_pool(name="idpool", bufs=1) as idp:
                irow = idp.tile([128, 128], F32, tag="irow")
                icol = idp.tile([128, 1], F32, tag="icol")
                nc.gpsimd.iota(irow[:], pattern=[[1, 128]], base=0,
                               channel_multiplier=0,
                               allow_small_or_imprecise_dtypes=True)
                nc.gpsimd.iota(icol[:], pattern=[[0, 1]], base=0,
                               channel_multiplier=1,
                               allow_small_or_imprecise_dtypes=True)
                nc.vector.tensor_scalar(idn[:], irow[:], icol[:], None, EQ)

            # ones columns of v (denominator accumulators)
            for h in range(H):
                nc.vector.memset(v_sb[:, :, h * VW + D], 1.0)

            # ---------- helpers (psum pools bound below) ----------
            def ldq(m, eng):
                """Load gathered q^T for head-pair m (persistent)."""
                qt = qa.tile([128, N_CORES, QS], BF16, tag="qT2")
                eng.dma_start(qt[:], gq_view[:, m, :, :])
                return qt

            def kproj(psA, m, th, tok, wcol, chunks=((0, 512),)):
                """k^T i-block m over t half th, straight into kTh2."""
                for c0, c1 in chunks:
                    w = c1 - c0
                    ps = psA.tile([128, QS], F32, tag="ps")
                    for kk in range(32):
                        nc.tensor.matmul(
                            ps[:, 0:w], wcol[:, kk, :], tok[:, kk, c0:c1],
                            start=(kk == 0), stop=(kk == 31),
                        )
                    nc.vector.tensor_copy(
                        kTh2[:, m, th * 512 + c0:th * 512 + c1], ps[:, 0:w]
                    )

            def vproj(psA, psS, m, th, tok, wcol, stage):
                """v^T i-block m, t half th; transpose to v [t, i] on PE."""
                ps = psA.tile([128, QS], F32, tag="ps")
                for kk in range(32):
                    nc.tensor.matmul(
                        ps[:], wcol[:, kk, :], tok[:, kk, :],
                        start=(kk == 0), stop=(kk == 31),
                    )
                vst = stage.tile([128, QS], BF16, tag="vst")
                nc.vector.tensor_copy(vst[:], ps[:])
                pt = psS.tile([128, QS], BF16, tag="ss")
                for j in range(4):
                    nc.tensor.transpose(
                        pt[:, j * 128:(j + 1) * 128],
                        vst[:, j * 128:(j + 1) * 128],
                        idn[:],
                    )
                # pt[t, (j hh dd)] -> v_sb[t, th*4+j, (2m+hh)*VW + dd]
                dst = v_sb[
                    :, th * 4:(th + 1) * 4, 2 * m * VW:(2 * m + 2) * VW
                ].rearrange("p j (hh w) -> p j hh w", hh=2)[:, :, :, 0:D]
                nc.vector.tensor_copy(
                    dst,
                    pt[:].rearrange("p (j hh w) -> p j hh w", j=4, hh=2),
                )

            def attn(psS, psacc, h, qt):
                """Attention for head h over the full local t shard, all
                queries; both t halves accumulate into one PSUM group;
                scores/exp/PV pipelined one group deep. Ends with the
                head's u store + ReduceScatter."""
                m, half = h // 2, h % 2
                p0 = 64 * half
                vh = v_sb[:, :, h * VW:(h + 1) * VW]
                pending = None
                for c in range(N_CORES):
                    acc = psacc.tile([128, 4, VW], F32, tag="acc")
                    for th in range(2):
                        for gi, g in enumerate(((0, 1), (2, 3))):
                            ss = psS.tile([128, 2, QS], F32, tag="ss")
                            for jj, j in enumerate(g):
                                jt = th * 4 + j
                                nc.tensor.matmul(
                                    ss[:, jj, :],
                                    kTh2[p0:p0 + 64, m, jt * 128:(jt + 1) * 128],
                                    qt[p0:p0 + 64, c, :],
                                    start=True, stop=True,
                                )
                            pT = pTp.tile([128, 2, QS], BF16, tag="pT")
                            nc.scalar.activation(pT[:], ss[:], EXP, scale=SCALE)
                            if pending is not None:
                                pending()
                                pending = None

                            def mk(g=g, pT=pT, acc=acc, c=c, th=th, gi=gi):
                                def pv():
                                    for jj, j in enumerate(g):
                                        for qq in range(4):
                                            nc.tensor.matmul(
                                                acc[:, qq, :],
                                                pT[:, jj, qq * 128:(qq + 1) * 128],
                                                vh[:, th * 4 + j, :],
                                                start=(th == 0 and gi == 0
                                                       and jj == 0 and qq == 0),
                                                stop=(th == 1 and gi == 1
                                                      and jj == 1 and qq == 3),
                                                skip_group_check=True,
                                            )
                                    if th == 1 and gi == 1:
                                        nc.vector.tensor_copy(
                                            u_loc[:, h % 2, c * 4:(c + 1) * 4, :],
                                            acc[:],
                                        )
                                return pv

                            pending = mk()
                pending()
                udst = u_dram[h].rearrange("(cq p) w -> p cq w", p=128)
                nc.sync.dma_start(udst, u_loc[:, h % 2])
                if _NO_CC:
                    nc.sync.dma_start(u_red[h], u_dram[h, 0:QS, :])
                else:
                    nc.gpsimd.collective_compute(
                        "ReduceScatter", ADD,
                        replica_groups=[list(range(N_CORES))],
                        ins=[u_dram[h].opt()], outs=[u_red[h].opt()],
                    )

            def fin(h, tpool, ttag="ss"):
                """Normalize head h's reduced sums into aT_sb."""
                fint = finp.tile([128, 4, VW], BF16, tag="fin")
                nc.scalar.dma_start(
                    fint[:], u_red[h].rearrange("(qq p) w -> p qq w", p=128)
                )
                rec = finp.tile([128, 4], F32, tag="rec")
                nc.vector.reciprocal(rec[:], fint[:, :, D])
                an = finp.tile([128, 4, D], BF16, tag="an")
                for qq in range(4):
                    nc.vector.tensor_scalar(
                        an[:, qq, :], fint[:, qq, 0:D], rec[:, qq:qq + 1],
                        None, MULT,
                    )
                pt_a = tpool.tile([64, 512], BF16, tag=ttag)
                for qq in range(4):
                    nc.tensor.transpose(
                        pt_a[:, qq * 128:(qq + 1) * 128], an[:, qq, :], idn[:]
                    )
                nc.vector.tensor_copy(
                    aT_sb[(h % 2) * 64:(h % 2) * 64 + 64, h // 2, :], pt_a[:]
                )

            # ================ main schedule ================
            with (
                tc.tile_pool(name="psS", bufs=2, space="PSUM") as psS,
                tc.tile_pool(name="psacc", bufs=2, space="PSUM") as psacc,
                tc.tile_pool(name="psA", bufs=2, space="PSUM") as psA,
            ):
                with (
                    tc.tile_pool(name="tokp", bufs=1) as tokp,
                    tc.tile_pool(name="wpool", bufs=4) as wpool,
                    tc.tile_pool(name="stage", bufs=2) as stage,
                ):
                    w_views = {"k": w_k_v, "v": w_v_v}

                    def ldw(kind, m, eng):
                        wcol = wpool.tile([128, 32, 128], BF16, tag="wcol")
                        eng.dma_start(
                            wcol[:], w_views[kind][:, :, m * 128:(m + 1) * 128]
                        )
                        return wcol

                    # ---- q projection + gather ----
                    with tc.tile_pool(name="qp", bufs=1) as qp:
                        wq_sb = qp.tile([128, 4, INNER], BF16, tag="wq")
                        lq_sb = qp.tile([128, 4, QS], BF16, tag="lq")
                        nc.scalar.dma_start(lq_sb[:], lq_v)
                        for mq in range(4):
                            nc.scalar.dma_start(
                                wq_sb[:, :, mq * 128:(mq + 1) * 128],
                                w_q_v[:, :, mq * 128:(mq + 1) * 128],
                            )
                        tok0 = tokp.tile([128, 32, 512], BF16, tag="tok")
                        nc.sync.dma_start(tok0[:, :, 0:256], tok_v[:, :, 0:256])
                        nc.sync.dma_start(tok0[:, :, 256:512],
                                          tok_v[:, :, 256:512])
                        for mq in range(4):
                            ps = psA.tile([128, QS], F32, tag="ps")
                            for kk in range(4):
                                nc.tensor.matmul(
                                    ps[:],
                                    wq_sb[:, kk, mq * 128:(mq + 1) * 128],
                                    lq_sb[:, kk, :],
                                    start=(kk == 0), stop=(kk == 3),
                                )
                            qst = stage.tile([128, QS], BF16, tag="vst")
                            nc.vector.tensor_copy(qst[:], ps[:])
                            nc.scalar.dma_start(
                                gq_in[mq * 128:(mq + 1) * 128, :], qst[:]
                            )
                        if _NO_CC:
                            nc.sync.dma_start(gq_out[0:INNER, :], gq_in[:])
                        else:
                            nc.gpsimd.collective_compute(
                                "AllGather", mybir.AluOpType.bypass,
                                replica_groups=[list(range(N_CORES))],
                                ins=[gq_in.opt()], outs=[gq_out.opt()],
                            )

                    # ---- projections + attention, per head-pair ----
                    wk = ldw("k", 0, nc.scalar)
                    wv = ldw("v", 0, nc.vector)
                    kproj(psA, 0, 0, tok0, wk, chunks=((0, 256), (256, 512)))
                    vproj(psA, psS, 0, 0, tok0, wv, stage)
                    wkb = ldw("k", 0, nc.sync)
                    wvb = ldw("v", 0, nc.vector)
                    kproj(psA, 0, 1, tok1, wkb)
                    vproj(psA, psS, 0, 1, tok1, wvb, stage)
                    qts = [ldq(0, nc.vector)]
                    wk = ldw("k", 1, nc.scalar)
                    wv = ldw("v", 1, nc.vector)
                    kproj(psA, 1, 0, tok0, wk)
                    vproj(psA, psS, 1, 0, tok0, wv, stage)
                    attn(psS, psacc, 0, qts[0])
                    wkb = ldw("k", 1, nc.sync)
                    wvb = ldw("v", 1, nc.vector)
                    kproj(psA, 1, 1, tok1, wkb)
                    vproj(psA, psS, 1, 1, tok1, wvb, stage)
                    attn(psS, psacc, 1, qts[0])
                    qts.append(ldq(1, nc.vector))
                    wk = ldw("k", 2, nc.scalar)
                    wv = ldw("v", 2, nc.vector)
                    kproj(psA, 2, 0, tok0, wk)
                    vproj(psA, psS, 2, 0, tok0, wv, stage)
                    attn(psS, psacc, 2, qts[1])
                    wkb = ldw("k", 2, nc.sync)
                    wvb = ldw("v", 2, nc.vector)
                    kproj(psA, 2, 1, tok1, wkb)
                    vproj(psA, psS, 2, 1, tok1, wvb, stage)
                    attn(psS, psacc, 3, qts[1])
                    qts.append(ldq(2, nc.vector))
                    wk = ldw("k", 3, nc.sync)
                    wv = ldw("v", 3, nc.vector)
                    kproj(psA, 3, 0, tok0, wk)
                    vproj(psA, psS, 3, 0, tok0, wv, stage)
                    attn(psS, psacc, 4, qts[2])
                    wkb = ldw("k", 3, nc.sync)
                    wvb = ldw("v", 3, nc.vector)
                    kproj(psA, 3, 1, tok1, wkb)
                    vproj(psA, psS, 3, 1, tok1, wvb, stage)
                    attn(psS, psacc, 5, qts[2])
                    qts.append(ldq(3, nc.vector))

                # tok/w pools closed: preload w_out (sync HWDGE; the Pool
                # queue blocks on in-flight collectives)
                nc.sync.dma_start(wo_all[:], w_out_v)
                attn(psS, psacc, 6, qts[3])
                attn(psS, psacc, 7, qts[3])

            # ---------------- finalize + output projection --------
            # Pass A (kk 0-2, heads 0-5) runs while the last ReduceScatters
            # are in flight; pass B adds the kk=3 contribution once heads
            # 6/7 are finalized. Partials stay f32 in SBUF.
            with (
                tc.tile_pool(name="outps", bufs=4, space="PSUM") as outps,
                tc.tile_pool(name="outp", bufs=3) as outp,
                tc.tile_pool(name="poutp", bufs=1) as poutp,
            ):
                pout = poutp.tile([128, L // 128, QS], F32, tag="pout")
                with tc.tile_wait_until(1.0):
                    for h in range(6):
                        fin(h, outps, "po")
                    for mo in range(L // 128):
                        ps = outps.tile([128, QS], F32, tag="po")
                        for kk in range(3):
                            nc.tensor.matmul(
                                ps[:],
                                wo_all[:, kk, mo * 128:(mo + 1) * 128],
                                aT_sb[:, kk, :],
                                start=(kk == 0), stop=(kk == 2),
                            )
                        nc.vector.tensor_copy(pout[:, mo, :], ps[:])
                    fin(6, outps, "po")
                    fin(7, outps, "po")
                    for mo in range(L // 128):
                        ps = outps.tile([128, QS], F32, tag="po")
                        nc.tensor.matmul(
                            ps[:],
                            wo_all[:, 3, mo * 128:(mo + 1) * 128],
                            aT_sb[:, 3, :],
                            start=True, stop=True,
                        )
                        of = outp.tile([128, QS], F32, tag="of")
                        nc.vector.tensor_add(of[:], ps[:], pout[:, mo, :])
                        nc.sync.dma_start(
                            outT[mo * 128:(mo + 1) * 128, :], of[:]
                        )

    nc.compile()
    return nc


_COMPILED = None


def _get_compiled():
    global _COMPILED
    if _COMPILED is None:
        _COMPILED = build_program()
    return _COMPILED


def make_in_maps(token_input, learned_queries, w_q, w_k, w_v, w_out):
    import ml_dtypes

    bf16 = ml_dtypes.bfloat16
    token_input = np.asarray(token_input, dtype=np.float32).astype(bf16)
    learned_queries = np.asarray(learned_queries, dtype=np.float32).astype(bf16)
    w_q = np.ascontiguousarray(np.asarray(w_q, dtype=np.float32).astype(bf16))
    w_k = np.ascontiguousarray(np.asarray(w_k, dtype=np.float32).astype(bf16))
    w_v = np.ascontiguousarray(np.asarray(w_v, dtype=np.float32).astype(bf16))
    w_out = np.ascontiguousarray(np.asarray(w_out, dtype=np.float32).astype(bf16))
    in_maps = []
    for c in range(N_CORES):
        in_maps.append({
            "tok_T": np.ascontiguousarray(token_input[c * TS:(c + 1) * TS, :].T),
            "lq_T": np.ascontiguousarray(learned_queries[c * QS:(c + 1) * QS, :].T),
            "w_q": w_q, "w_k": w_k, "w_v": w_v, "w_out": w_out,
        })
    return in_maps


def assemble(results):
    out = np.empty((V, L), dtype=np.float32)
    for c in range(N_CORES):
        out[c * QS:(c + 1) * QS, :] = results[c]["outT"].T
    return out


def kernel(token_input, learned_queries, w_q, w_k, w_v, w_out):
    nc = _get_compiled()
    in_maps = make_in_maps(token_input, learned_queries, w_q, w_k, w_v, w_out)
    res = run_bass_kernel_spmd(nc, in_maps, list(range(N_CORES)))
    return assemble(res.results)


# revision 6
# speedup vs baseline: 1.3618x; 1.3618x over previous
"""Cross-attention reducer kernel for Trainium2, 8 NeuronCores (SPMD), v3.

Problem (full shapes):
    token_input    [T=8192, L=4096]
    learned_queries[V=4096, I=512]
    w_q [I, I], w_k [L, I], w_v [L, I], w_out [I, L]

    q = learned_queries @ w_q;  k = token_input @ w_k;  v = token_input @ w_v
    per head h (H=8, D=64): attn = softmax(q_h k_h^T / sqrt(D)); out_h = attn @ v_h
    out = concat_h(out_h) @ w_out      -> [V, L]

Sharding: sequence-parallel attention with an all-gather of q and per-head
ReduceScatters of partial softmax sums (flash-style partial-sum reduction).
Each core projects its T/8 token shard with the full w_k/w_v, computes for
ALL 4096 queries x 8 heads the partial numerator sum exp(s) v and partial
denominator sum exp(s) over its local tokens (exp without max subtraction:
logits are O(3)), and per head ReduceScatter-sums the partials so core r
gets its own V/8 query block fully reduced; it normalizes and applies w_out.

v3 changes over the earlier kernel:
  - all inputs host-cast to bf16 (halves HBM traffic; matmul error stays
    ~1e-3, far under the 2e-2 gate); projections run bf16 x bf16.
  - head-pair packing: k^T kept as [128 = two heads' D rows, t]; the odd
    head's score matmul uses base partition 64 (PE tile_position (64,0)),
    removing all k partition-shift DMAs; q^T gathered tiles are loaded once
    per head-pair (persistent, 4 tiles) instead of twice per head.
  - partial u accumulated in SBUF (bf16): th0 drains PSUM->u_loc, th1 adds
    PSUM+u_loc->u_loc; removes the DRAM accumulate-DMA bounce entirely.
    ReduceScatters run on bf16 (half the bytes, earlier completion).
  - per-head finalize as soon as its ReduceScatter lands; the last head's
    reduction is the only exposed one.
  - scores/exp/PV software-pipelined one group deep (PE never sits on an
    exp dependency with an empty queue); exp is the only ACT-engine work.
  - startup: lq/wq land first (q proj), first k-projection runs on t-quarter
    chunks as the token shard streams in.
  - w_out preloaded whole (32KB bf16) mid-kernel; output projection drains
    through 4 PSUM banks back-to-back.
"""

import os

import numpy as np

import concourse.bacc as bacc
import concourse.tile as tile
import concourse.mybir as mybir
from concourse.bass_utils import run_bass_kernel_spmd

F32 = mybir.dt.float32
BF16 = mybir.dt.bfloat16
EXP = mybir.ActivationFunctionType.Exp
MULT = mybir.AluOpType.mult
ADD = mybir.AluOpType.add
EQ = mybir.AluOpType.is_equal

N_CORES = 8
T, L, V, INNER = 8192, 4096, 4096, 512
H, D = 8, 64
M = H // 2             # 4 head-pairs == 128-wide i-blocks
TS = T // N_CORES      # 1024  t-shard per core
QS = V // N_CORES      # 512   query shard per core (= ReduceScatter block)
SCALE = D ** -0.5      # 0.125
VW = D + 1             # 65    v head block width incl. ones column

# diagnostics: BASSK_NO_CC=1 -> replace collectives with local copies
# (wrong data, timing only)
_NO_CC = bool(os.environ.get("BASSK_NO_CC"))


def build_program():
    nc = bacc.Bacc(
        "TRN2", target_bir_lowering=False, debug=False, num_devices=N_CORES
    )

    tok_T = nc.dram_tensor("tok_T", [L, TS], BF16, kind="ExternalInput").ap()
    lq_T = nc.dram_tensor("lq_T", [INNER, QS], BF16, kind="ExternalInput").ap()
    w_q = nc.dram_tensor("w_q", [INNER, INNER], BF16, kind="ExternalInput").ap()
    w_k = nc.dram_tensor("w_k", [L, INNER], BF16, kind="ExternalInput").ap()
    w_v = nc.dram_tensor("w_v", [L, INNER], BF16, kind="ExternalInput").ap()
    w_out = nc.dram_tensor("w_out", [INNER, L], BF16, kind="ExternalInput").ap()
    outT = nc.dram_tensor("outT", [L, QS], F32, kind="ExternalOutput").ap()

    # partition-major DRAM views
    tok_v = tok_T.rearrange("(k p) t -> p k t", p=128)      # [128, 32, 1024]
    lq_v = lq_T.rearrange("(k p) q -> p k q", p=128)        # [128, 4, 512]
    w_q_v = w_q.rearrange("(k p) i -> p k i", p=128)        # [128, 4, 512]
    w_k_v = w_k.rearrange("(k p) i -> p k i", p=128)        # [128, 32, 512]
    w_v_v = w_v.rearrange("(k p) i -> p k i", p=128)        # [128, 32, 512]
    w_out_v = w_out.rearrange("(k p) l -> p k l", p=128)    # [128, 4, 4096]

    with tile.TileContext(nc) as tc:
        with (
            tc.tile_pool(name="persist", bufs=1) as persist,
            tc.tile_pool(name="qa", bufs=2) as qa,
            tc.tile_pool(name="pTp", bufs=3) as pTp,
            tc.tile_pool(name="finp", bufs=2) as finp,
            tc.tile_pool(name="wop", bufs=1) as wop,
            tc.tile_pool(name="dram", bufs=1, space="DRAM") as dram,
        ):
            # ---- persistent SBUF ----
            kTh2 = persist.tile([128, M, TS], BF16, tag="kTh2")      # k^T head-pairs
            v_sb = persist.tile([128, TS // 128, H * VW], BF16, tag="v")
            u_loc = persist.tile([128, 2, 32, VW], BF16, tag="u")    # store staging
            aT_sb = persist.tile([128, 4, QS], BF16, tag="aT")       # attn out^T
            idn = persist.tile([128, 128], BF16, tag="idn")          # PE transpose id
            wo_all = wop.tile([128, 4, L], BF16, tag="wo")           # w_out (bf16)

            # collective buffers
            gq_in = dram.tile([INNER, QS], BF16, tag="gq_in")
            gq_out = dram.tile(
                [N_CORES * INNER, QS], BF16, tag="gq_out", addr_space="Shared"
            )
            u_dram = dram.tile([H, V, VW], BF16, tag="u_dram")
            u_red = dram.tile([H, QS, VW], BF16, tag="u_red")

            # gathered q^T viewed per head-pair: [128, m, c, q]
            gq_view = gq_out.rearrange("(c m p) q -> p m c q", p=128, m=M)

            # identity matrix for PE transposes: idn[p, f] = (f == p)
            with tc.tile_pool(name="idpool", bufs=1) as idp:
                irow = idp.tile([128, 128], F32, tag="irow")
                icol = idp.tile([128, 1], F32, tag="icol")
                nc.gpsimd.iota(irow[:], pattern=[[1, 128]], base=0,
                               channel_multiplier=0,
                               allow_small_or_imprecise_dtypes=True)
                nc.gpsimd.iota(icol[:], pattern=[[0, 1]], base=0,
                               channel_multiplier=1,
                               allow_small_or_imprecise_dtypes=True)
                nc.vector.tensor_scalar(idn[:], irow[:], icol[:], None, EQ)

            # ones columns of v (denominator accumulators)
            for h in range(H):
                nc.vector.memset(v_sb[:, :, h * VW + D], 1.0)

            # ---------- helpers (psum pools bound below) ----------
            def ldq(m, eng):
                """Load gathered q^T for head-pair m (persistent)."""
                qt = qa.tile([128, N_CORES, QS], BF16, tag="qT2")
                eng.dma_start(qt[:], gq_view[:, m, :, :])
                return qt

            def kproj(psA, m, th, tok, wcol, chunks=((0, 512),)):
                """k^T i-block m over t half th, straight into kTh2."""
                for c0, c1 in chunks:
                    w = c1 - c0
                    ps = psA.tile([128, QS], F32, tag="ps")
                    for kk in range(32):
                        nc.tensor.matmul(
                            ps[:, 0:w], wcol[:, kk, :], tok[:, kk, c0:c1],
                            start=(kk == 0), stop=(kk == 31),
                        )
                    nc.vector.tensor_copy(
                        kTh2[:, m, th * 512 + c0:th * 512 + c1], ps[:, 0:w]
                    )

            def vproj(psA, psS, m, th, tok, wcol, stage):
                """v^T i-block m, t half th; transpose to v [t, i] on PE."""
                ps = psA.tile([128, QS], F32, tag="ps")
                for kk in range(32):
                    nc.tensor.matmul(
                        ps[:], wcol[:, kk, :], tok[:, kk, :],
                        start=(kk == 0), stop=(kk == 31),
                    )
                vst = stage.tile([128, QS], BF16, tag="vst")
                nc.vector.tensor_copy(vst[:], ps[:])
                pt = psS.tile([128, QS], BF16, tag="ss")
                for j in range(4):
                    nc.tensor.transpose(
                        pt[:, j * 128:(j + 1) * 128],
                        vst[:, j * 128:(j + 1) * 128],
                        idn[:],
                    )
                # pt[t, (j hh dd)] -> v_sb[t, th*4+j, (2m+hh)*VW + dd]
                dst = v_sb[
                    :, th * 4:(th + 1) * 4, 2 * m * VW:(2 * m + 2) * VW
                ].rearrange("p j (hh w) -> p j hh w", hh=2)[:, :, :, 0:D]
                nc.vector.tensor_copy(
                    dst,
                    pt[:].rearrange("p (j hh w) -> p j hh w", j=4, hh=2),
                )

            def attn(psS, psacc, h, qt):
                """Attention for head h over the full local t shard, all
                queries; both t halves accumulate into one PSUM group;
                scores/exp/PV pipelined one group deep. Ends with the
                head's u store + ReduceScatter."""
                m, half = h // 2, h % 2
                p0 = 64 * half
                vh = v_sb[:, :, h * VW:(h + 1) * VW]
                pending = None
                for c in range(N_CORES):
                    acc = psacc.tile([128, 4, VW], F32, tag="acc")
                    for th in range(2):
                        for gi, g in enumerate(((0, 1), (2, 3))):
                            ss = psS.tile([128, 2, QS], F32, tag="ss")
                            for jj, j in enumerate(g):
                                jt = th * 4 + j
                                nc.tensor.matmul(
                                    ss[:, jj, :],
                                    kTh2[p0:p0 + 64, m, jt * 128:(jt + 1) * 128],
                                    qt[p0:p0 + 64, c, :],
                                    start=True, stop=True,
                                )
                            pT = pTp.tile([128, 2, QS], BF16, tag="pT")
                            nc.scalar.activation(pT[:], ss[:], EXP, scale=SCALE)
                            if pending is not None:
                                pending()
                                pending = None

                            def mk(g=g, pT=pT, acc=acc, c=c, th=th, gi=gi):
                                def pv():
                                    for jj, j in enumerate(g):
                                        for qq in range(4):
                                            nc.tensor.matmul(
                                                acc[:, qq, :],
                                                pT[:, jj, qq * 128:(qq + 1) * 128],
                                                vh[:, th * 4 + j, :],
                                                start=(th == 0 and gi == 0
                                                       and jj == 0 and qq == 0),
                                                stop=(th == 1 and gi == 1
                                                      and jj == 1 and qq == 3),
                                                skip_group_check=True,
                                            )
                                    if th == 1 and gi == 1:
                                        nc.vector.tensor_copy(
                                            u_loc[:, h % 2, c * 4:(c + 1) * 4, :],
                                            acc[:],
                                        )
                                        udst = u_dram[h].rearrange(
                                            "(cq p) w -> p cq w", p=128
                                        )[:, c * 4:(c + 1) * 4, :]
                                        nc.sync.dma_start(
                                            udst,
                                            u_loc[:, h % 2, c * 4:(c + 1) * 4, :],
                                        )
                                return pv

                            pending = mk()
                pending()
                if _NO_CC:
                    nc.sync.dma_start(u_red[h], u_dram[h, 0:QS, :])
                else:
                    nc.gpsimd.collective_compute(
                        "ReduceScatter", ADD,
                        replica_groups=[list(range(N_CORES))],
                        ins=[u_dram[h].opt()], outs=[u_red[h].opt()],
                    )

            def fin(h, tpool, ttag="ss"):
                """Normalize head h's reduced sums into aT_sb."""
                fint = finp.tile([128, 4, VW], BF16, tag="fin")
                nc.scalar.dma_start(
                    fint[:], u_red[h].rearrange("(qq p) w -> p qq w", p=128)
                )
                rec = finp.tile([128, 4], F32, tag="rec")
                nc.vector.reciprocal(rec[:], fint[:, :, D])
                an = finp.tile([128, 4, D], BF16, tag="an")
                for qq in range(4):
                    nc.vector.tensor_scalar(
                        an[:, qq, :], fint[:, qq, 0:D], rec[:, qq:qq + 1],
                        None, MULT,
                    )
                pt_a = tpool.tile([64, 512], BF16, tag=ttag)
                for qq in range(4):
                    nc.tensor.transpose(
                        pt_a[:, qq * 128:(qq + 1) * 128], an[:, qq, :], idn[:]
                    )
                nc.vector.tensor_copy(
                    aT_sb[(h % 2) * 64:(h % 2) * 64 + 64, h // 2, :], pt_a[:]
                )

            # ================ main schedule ================
            with (
                tc.tile_pool(name="psS", bufs=2, space="PSUM") as psS,
                tc.tile_pool(name="psacc", bufs=2, space="PSUM") as psacc,
                tc.tile_pool(name="psA", bufs=2, space="PSUM") as psA,
            ):
                with (
                    tc.tile_pool(name="tokp", bufs=1) as tokp,
                    tc.tile_pool(name="wpool", bufs=4) as wpool,
                    tc.tile_pool(name="stage", bufs=2) as stage,
                ):
                    w_views = {"k": w_k_v, "v": w_v_v}

                    def ldw(kind, m, eng):
                        wcol = wpool.tile([128, 32, 128], BF16, tag="wcol")
                        eng.dma_start(
                            wcol[:], w_views[kind][:, :, m * 128:(m + 1) * 128]
                        )
                        return wcol

                    # ---- q projection + gather ----
                    with tc.tile_pool(name="qp", bufs=1) as qp:
                        wq_sb = qp.tile([128, 4, INNER], BF16, tag="wq")
                        lq_sb = qp.tile([128, 4, QS], BF16, tag="lq")
                        nc.scalar.dma_start(lq_sb[:], lq_v)
                        for mq in range(4):
                            nc.scalar.dma_start(
                                wq_sb[:, :, mq * 128:(mq + 1) * 128],
                                w_q_v[:, :, mq * 128:(mq + 1) * 128],
                            )
                        tok0 = tokp.tile([128, 32, 512], BF16, tag="tok")
                        nc.sync.dma_start(tok0[:, :, 0:256], tok_v[:, :, 0:256])
                        nc.sync.dma_start(tok0[:, :, 256:512],
                                          tok_v[:, :, 256:512])
                        for mq in range(4):
                            ps = psA.tile([128, QS], F32, tag="ps")
                            for kk in range(4):
                                nc.tensor.matmul(
                                    ps[:],
                                    wq_sb[:, kk, mq * 128:(mq + 1) * 128],
                                    lq_sb[:, kk, :],
                                    start=(kk == 0), stop=(kk == 3),
                                )
                            qst = stage.tile([128, QS], BF16, tag="vst")
                            nc.vector.tensor_copy(qst[:], ps[:])
                            nc.scalar.dma_start(
                                gq_in[mq * 128:(mq + 1) * 128, :], qst[:]
                            )
                        if _NO_CC:
                            nc.sync.dma_start(gq_out[0:INNER, :], gq_in[:])
                        else:
                            nc.gpsimd.collective_compute(
                                "AllGather", mybir.AluOpType.bypass,
                                replica_groups=[list(range(N_CORES))],
                                ins=[gq_in.opt()], outs=[gq_out.opt()],
                            )

                    # ---- projections + attention, per head-pair ----
                    wk = ldw("k", 0, nc.scalar)
                    wv = ldw("v", 0, nc.vector)
                    kproj(psA, 0, 0, tok0, wk, chunks=((0, 256), (256, 512)))
                    vproj(psA, psS, 0, 0, tok0, wv, stage)
                    wkb = ldw("k", 0, nc.sync)
                    wvb = ldw("v", 0, nc.vector)
                    kproj(psA, 0, 1, tok1, wkb)
                    vproj(psA, psS, 0, 1, tok1, wvb, stage)
                    qts = [ldq(0, nc.vector)]
                    wk = ldw("k", 1, nc.scalar)
                    wv = ldw("v", 1, nc.vector)
                    kproj(psA, 1, 0, tok0, wk)
                    vproj(psA, psS, 1, 0, tok0, wv, stage)
                    attn(psS, psacc, 0, qts[0])
                    wkb = ldw("k", 1, nc.sync)
                    wvb = ldw("v", 1, nc.vector)
                    kproj(psA, 1, 1, tok1, wkb)
                    vproj(psA, psS, 1, 1, tok1, wvb, stage)
                    attn(psS, psacc, 1, qts[0])
                    qts.append(ldq(1, nc.vector))
                    wk = ldw("k", 2, nc.scalar)
                    wv = ldw("v", 2, nc.vector)
                    kproj(psA, 2, 0, tok0, wk)
                    vproj(psA, psS, 2, 0, tok0, wv, stage)
                    attn(psS, psacc, 2, qts[1])
                    wkb = ldw("k", 2, nc.sync)
                    wvb = ldw("v", 2, nc.vector)
                    kproj(psA, 2, 1, tok1, wkb)
                    vproj(psA, psS, 2, 1, tok1, wvb, stage)
                    attn(psS, psacc, 3, qts[1])
                    qts.append(ldq(2, nc.vector))
                    wk = ldw("k", 3, nc.sync)
                    wv = ldw("v", 3, nc.vector)
                    kproj(psA, 3, 0, tok0, wk)
                    vproj(psA, psS, 3, 0, tok0, wv, stage)
                    attn(psS, psacc, 4, qts[2])
                    wkb = ldw("k", 3, nc.sync)
                    wvb = ldw("v", 3, nc.vector)
                    kproj(psA, 3, 1, tok1, wkb)
                    vproj(psA, psS, 3, 1, tok1, wvb, stage)
                    attn(psS, psacc, 5, qts[2])
                    qts.append(ldq(3, nc.vector))

                # tok/w pools closed: preload w_out (sync HWDGE; the Pool
                # queue blocks on in-flight collectives)
                nc.sync.dma_start(wo_all[:], w_out_v)
                attn(psS, psacc, 6, qts[3])
                attn(psS, psacc, 7, qts[3])

            # ---------------- finalize + output projection --------
            # Pass A (kk 0-2, heads 0-5) runs while the last ReduceScatters
            # are in flight; pass B adds the kk=3 contribution once heads
            # 6/7 are finalized. Partials stay f32 in SBUF.
            with (
                tc.tile_pool(name="outps", bufs=4, space="PSUM") as outps,
                tc.tile_pool(name="outp", bufs=3) as outp,
                tc.tile_pool(name="poutp", bufs=1) as poutp,
            ):
                pout = poutp.tile([128, L // 128, QS], F32, tag="pout")
                with tc.tile_wait_until(1.0):
                    for h in range(6):
                        fin(h, outps, "po")
                    for mo in range(L // 128):
                        ps = outps.tile([128, QS], F32, tag="po")
                        for kk in range(3):
                            nc.tensor.matmul(
                                ps[:],
                                wo_all[:, kk, mo * 128:(mo + 1) * 128],
                                aT_sb[:, kk, :],
                                start=(kk == 0), stop=(kk == 2),
                            )
                        nc.vector.tensor_copy(pout[:, mo, :], ps[:])
                    fin(6, outps, "po")
                    fin(7, outps, "po")
                    for mo in range(L // 128):
                        ps = outps.tile([128, QS], F32, tag="po")
                        nc.tensor.matmul(
                            ps[:],
                            wo_all[:, 3, mo * 128:(mo + 1) * 128],
                            aT_sb[:, 3, :],
                            start=True, stop=True,
                        )
                        of = outp.tile([128, QS], F32, tag="of")
                        nc.vector.tensor_add(of[:], ps[:], pout[:, mo, :])
                        nc.sync.dma_start(
                            outT[mo * 128:(mo + 1) * 128, :], of[:]
                        )

    nc.compile()
    return nc


_COMPILED = None


def _get_compiled():
    global _COMPILED
    if _COMPILED is None:
        _COMPILED = build_program()
    return _COMPILED


def make_in_maps(token_input, learned_queries, w_q, w_k, w_v, w_out):
    import ml_dtypes

    bf16 = ml_dtypes.bfloat16
    token_input = np.asarray(token_input, dtype=np.float32).astype(bf16)
    learned_queries = np.asarray(learned_queries, dtype=np.float32).astype(bf16)
    w_q = np.ascontiguousarray(np.asarray(w_q, dtype=np.float32).astype(bf16))
    w_k = np.ascontiguousarray(np.asarray(w_k, dtype=np.float32).astype(bf16))
    w_v = np.ascontiguousarray(np.asarray(w_v, dtype=np.float32).astype(bf16))
    w_out = np.ascontiguousarray(np.asarray(w_out, dtype=np.float32).astype(bf16))
    in_maps = []
    for c in range(N_CORES):
        in_maps.append({
            "tok_T": np.ascontiguousarray(token_input[c * TS:(c + 1) * TS, :].T),
            "lq_T": np.ascontiguousarray(learned_queries[c * QS:(c + 1) * QS, :].T),
            "w_q": w_q, "w_k": w_k, "w_v": w_v, "w_out": w_out,
        })
    return in_maps


def assemble(results):
    out = np.empty((V, L), dtype=np.float32)
    for c in range(N_CORES):
        out[c * QS:(c + 1) * QS, :] = results[c]["outT"].T
    return out


def kernel(token_input, learned_queries, w_q, w_k, w_v, w_out):
    nc = _get_compiled()
    in_maps = make_in_maps(token_input, learned_queries, w_q, w_k, w_v, w_out)
    res = run_bass_kernel_spmd(nc, in_maps, list(range(N_CORES)))
    return assemble(res.results)
